# revision 12
# baseline (speedup 1.0000x reference)
"""GATv2 message-passing kernel for 8 Trainium2 NeuronCores.

Strategy (per core; targets sharded by node range, edge routing on-chip):
  - Host: index-only preprocessing. Targets degree-sorted into rank order;
    edges laid out as [128 target-partitions x T slots] with chunk-uniform
    slot widths. Source xl values are routed on-chip (no per-edge DMA):
      ap_gather expansion (GPSIMD) -> local_scatter into a
      (src-partition x dst-partition) bucket grid -> blocked DMA-XBAR
      transpose -> local_scatter into the target-major grid.
  - Device: xl/xr via PE matmuls on the core's rank-permuted node shard;
    bf16 xl table AllGathered; alpha/softmax/numerators on DVE+ACT with
    chunk-wide 3D-AP ops; per-target segment stats along the free axis.
"""

import numpy as np

N_NODES = 100000
N_EDGES = 6400000
D_IN = 256
OUT = 2
NEG_SLOPE = 0.2
N_CORES = 8
NPC = N_NODES // N_CORES
NT = 98
NPAD = NT * 128
NTAB = NPAD * N_CORES
VPP = NTAB // 128
CLS_W = 1920
WIN_H = 15
NTC = 7

_CACHE = {}


def _host_prep(x, edge_index, edge_attr, Wl, bl, Wr, br, We, att, bias):
    import ml_dtypes

    import hostprep as hp

    src = np.asarray(edge_index[0], dtype=np.int64)
    tgt = np.asarray(edge_index[1], dtype=np.int64)
    ea = np.asarray(edge_attr, dtype=np.float32).reshape(-1)
    x = np.asarray(x, dtype=np.float32)

    maps, meta, node_perm, rank_of = hp.prep(src, tgt, ea)

    bf16 = ml_dtypes.bfloat16
    xT = np.ascontiguousarray(x.T)  # [256, N]

    W4 = np.concatenate([np.asarray(Wl, np.float32), np.asarray(Wr, np.float32)], axis=1)
    w4_sb = np.concatenate([W4[0:128, :], W4[128:256, :]], axis=1).astype(bf16)  # [128, 8]

    in_maps = []
    for k in range(N_CORES):
        # rank-permuted node columns: slab row r = node with rank r
        perm = node_perm[k]
        xTk = np.zeros((D_IN, NPAD), dtype=np.float32)
        valid = perm < NPC
        xTk[:, valid] = xT[:, k * NPC + perm[valid]]
        m = maps[k]
        im = {
            "xT": xTk.astype(bf16),
            "w4": w4_sb,
            "ea_pp": m["ea_pp"].astype(bf16),
            "mask_pp": m["mask_pp"].astype(bf16),
        }
        for w in range(meta["NW"]):
            im[f"apg{w}"] = m["apg_idx"][w]
            im[f"s1i{w}"] = m["s1_idx"][w]
        for j in range(meta["n_cls"]):
            im[f"s2i{j}"] = m["s2_idx"][j]
        in_maps.append(im)

    consts = dict(
        We0=float(np.asarray(We).reshape(-1)[0]), We1=float(np.asarray(We).reshape(-1)[1]),
        att0=float(np.asarray(att)[0]), att1=float(np.asarray(att)[1]),
        K0=float(np.asarray(bl)[0] + np.asarray(br)[0]),
        K1=float(np.asarray(bl)[1] + np.asarray(br)[1]),
        bl0=float(np.asarray(bl)[0]), bl1=float(np.asarray(bl)[1]),
        bias0=float(np.asarray(bias)[0]), bias1=float(np.asarray(bias)[1]),
    )
    return in_maps, consts, meta, node_perm


def _build(meta, consts):
    from concourse import bacc, mybir
    import concourse.bass as bass
    import concourse.tile as tile

    T = meta["T"]
    d_t = meta["d_t"]
    col_off = meta["col_off"]
    NW = meta["NW"]
    NI = meta["NI"]
    H = meta["H"]
    jbase = meta["jbase"]
    SH = meta["SH"]
    n_cls = meta["n_cls"]

    f32 = mybir.dt.float32
    bf16 = mybir.dt.bfloat16
    i16 = mybir.dt.int16
    AX = mybir.AxisListType.X
    ALU = mybir.AluOpType
    ACTF = mybir.ActivationFunctionType

    NImax = max(NI)
    Hmax = max(H)

    nc = bacc.Bacc("TRN2", target_bir_lowering=False, debug=False, num_devices=N_CORES,
                   dynamic_dma_scratch_size=8192)
    xT_d = nc.dram_tensor("xT", [D_IN, NPAD], bf16, kind="ExternalInput").ap()
    w4_d = nc.dram_tensor("w4", [128, 8], bf16, kind="ExternalInput").ap()
    ea_d = nc.dram_tensor("ea_pp", [128, T], bf16, kind="ExternalInput").ap()
    mask_d = nc.dram_tensor("mask_pp", [128, T], bf16, kind="ExternalInput").ap()
    apg_d = [nc.dram_tensor(f"apg{w}", [128, NI[w] // 16], i16, kind="ExternalInput").ap()
             for w in range(NW)]
    s1i_d = [nc.dram_tensor(f"s1i{w}", [128, 2 * NI[w]], i16, kind="ExternalInput").ap()
             for w in range(NW)]
    s2i_d = [nc.dram_tensor(f"s2i{j}", [128, H[j] * 128], i16, kind="ExternalInput").ap()
             for j in range(n_cls)]
    out_d = nc.dram_tensor("outp", [NPAD, OUT], f32, kind="ExternalOutput").ap()

    with tile.TileContext(nc) as tc:
        with tc.tile_pool(name="persist", bufs=1) as pp, \
             tc.tile_pool(name="stream", bufs=2) as sp, \
             tc.tile_pool(name="route", bufs=1) as rp, \
             tc.tile_pool(name="scratch", bufs=1) as scr, \
             tc.tile_pool(name="psum", bufs=2, space="PSUM") as psp, \
             tc.tile_pool(name="dram", bufs=1, space="DRAM") as dp:

            w4_sb = pp.tile([128, 8], bf16)
            nc.sync.dma_start(out=w4_sb[:], in_=w4_d[:])

            xl_slab_sb = pp.tile([128, 2 * NT], bf16)
            xr_sb = pp.tile([128, 2 * NT], f32)

            # ---- Phase A: xl/xr for this core's node shard (rank order) ----
            for nb in range(NT):
                xa = sp.tile([128, 128], bf16, tag="xa")
                xb = sp.tile([128, 128], bf16, tag="xb")
                nc.sync.dma_start(out=xa[:], in_=xT_d[0:128, 128 * nb:128 * (nb + 1)])
                nc.sync.dma_start(out=xb[:], in_=xT_d[128:256, 128 * nb:128 * (nb + 1)])
                ps = psp.tile([128, 4], f32)
                nc.tensor.matmul(out=ps[:], lhsT=xa[:], rhs=w4_sb[:, 0:4], start=True, stop=False)
                nc.tensor.matmul(out=ps[:], lhsT=xb[:], rhs=w4_sb[:, 4:8], start=False, stop=True)
                nc.vector.tensor_copy(out=xl_slab_sb[:, 2 * nb:2 * nb + 2], in_=ps[:, 0:2])
                nc.vector.tensor_copy(out=xr_sb[:, 2 * nb:2 * nb + 2], in_=ps[:, 2:4])

            xl_slab_d = dp.tile([NPAD, 2], bf16)
            nc.sync.dma_start(
                out=xl_slab_d[:].rearrange("(t p) c -> p t c", p=128),
                in_=xl_slab_sb[:].rearrange("p (t c) -> p t c", c=2),
            )

            xl_full = dp.tile([NTAB, 2], bf16)
            nc.gpsimd.collective_compute(
                "AllGather",
                mybir.AluOpType.bypass,
                replica_groups=[list(range(N_CORES))],
                ins=[xl_slab_d.opt()],
                outs=[xl_full.opt()],
            )
            tok = pp.tile([128, 2], bf16)
            nc.sync.dma_start(out=tok[:], in_=xl_full[NTAB - 128:NTAB, :])
            tc.strict_bb_all_engine_barrier()

            # xl table: partition q holds nodes [784q, 784(q+1)) interleaved ch
            tab_sb = pp.tile([128, 2 * VPP], bf16)
            nc.sync.dma_start(
                out=tab_sb[:].rearrange("q (v c) -> q v c", c=2),
                in_=xl_full[:].rearrange("(q v) c -> q v c", q=128),
            )

            # ---- Phase B/D: expansion + scatter#1 into bucket grid ----
            bucket = pp.tile([128, SH * 128], bf16)
            for w in range(NW):
                apg_sb = rp.tile([128, NImax // 16], i16, tag="apg")
                s1i_sb = rp.tile([128, 2 * NImax], i16, tag="s1i")
                nc.sync.dma_start(out=apg_sb[:, :NI[w] // 16], in_=apg_d[w][:])
                nc.sync.dma_start(out=s1i_sb[:, :2 * NI[w]], in_=s1i_d[w][:])
                exp_sb = rp.tile([128, 2 * NImax], bf16, tag="exp")
                nc.gpsimd.ap_gather(
                    out_ap=exp_sb[:, :2 * NI[w]].rearrange("p (i c) -> p i c", c=2),
                    in_ap=tab_sb[:].rearrange("q (v c) -> q v c", c=2),
                    idxs_ap=apg_sb[:, :NI[w] // 16],
                    channels=128,
                    num_elems=VPP,
                    d=2,
                    num_idxs=NI[w],
                )
                lo = WIN_H * w * 128
                ne = min(WIN_H * 128, SH * 128 - lo)
                nc.gpsimd.local_scatter(
                    out_ap=bucket[:, lo:lo + ne],
                    data_ap=exp_sb[:, :2 * NI[w]],
                    idxs_ap=s1i_sb[:, :2 * NI[w]],
                    channels=128,
                    num_elems=ne,
                    num_idxs=2 * NI[w],
                )

            # ---- Phase D'/E: transpose per class + scatter#2 into uF ----
            uF = pp.tile([128, 2 * T], bf16)
            for j in range(n_cls):
                tj = rp.tile([128, Hmax * 128], bf16, tag="tj")
                nc.sync.dma_start(
                    out=tj[:, :H[j] * 128].rearrange("p (b q) -> p b q", q=128),
                    in_=bucket[:, jbase[j] * 128:(jbase[j] + H[j]) * 128],
                    transpose=True,
                )
                s2i_sb = rp.tile([128, Hmax * 128], i16, tag="s2i")
                nc.sync.dma_start(out=s2i_sb[:, :H[j] * 128], in_=s2i_d[j][:])
                lo = CLS_W * j
                ne = min(CLS_W, 2 * T - lo)
                nc.gpsimd.local_scatter(
                    out_ap=uF[:, lo:lo + ne],
                    data_ap=tj[:, :H[j] * 128],
                    idxs_ap=s2i_sb[:, :H[j] * 128],
                    channels=128,
                    num_elems=ne,
                    num_idxs=H[j] * 128,
                )

            # ---- Phase C: alpha / softmax / numerators ----
            namax = pp.tile([128, NT], f32)
            denom = pp.tile([128, NT], f32)
            nume0 = pp.tile([128, NT], f32)
            nume1 = pp.tile([128, NT], f32)

            chunks = []
            for t0 in range(0, NT, NTC):
                t1 = min(t0 + NTC, NT)
                chunks.append((t0, t1, col_off[t0], col_off[t0] + (t1 - t0) * d_t[t0]))
            wmax = max(c[3] - c[2] for c in chunks)

            for (t0, t1, c0, c1) in chunks:
                Wc = c1 - c0
                ntc = t1 - t0
                d = d_t[t0]
                ea_c = sp.tile([128, wmax], bf16, tag="ea")
                mask_c = sp.tile([128, wmax], bf16, tag="mask")
                nc.sync.dma_start(out=ea_c[:, :Wc], in_=ea_d[:, c0:c1])
                nc.sync.dma_start(out=mask_c[:, :Wc], in_=mask_d[:, c0:c1])

                uch = uF[:, 2 * c0:2 * c1].rearrange("p (w c) -> p w c", c=2)
                u0 = uch[:, :, 0:1].rearrange("p w one -> p (w one)")
                u1 = uch[:, :, 1:2].rearrange("p w one -> p (w one)")

                v0 = scr.tile([128, wmax], f32, tag="v0")
                v1 = scr.tile([128, wmax], f32, tag="v1")
                nc.vector.tensor_scalar(out=v0[:, :Wc], in0=ea_c[:, :Wc],
                                        scalar1=consts["We0"], scalar2=consts["K0"],
                                        op0=ALU.mult, op1=ALU.add)
                nc.vector.tensor_scalar(out=v1[:, :Wc], in0=ea_c[:, :Wc],
                                        scalar1=consts["We1"], scalar2=consts["K1"],
                                        op0=ALU.mult, op1=ALU.add)
                nc.vector.tensor_tensor(out=v0[:, :Wc], in0=v0[:, :Wc], in1=u0, op=ALU.add)
                nc.vector.tensor_tensor(out=v1[:, :Wc], in0=v1[:, :Wc], in1=u1, op=ALU.add)

                # += xr (per-partition per-tile broadcast), chunk-wide 3D AP
                xr0 = xr_sb[:, 2 * t0:2 * t1].rearrange("p (t c) -> p t c", c=2)[:, :, 0:1]
                xr1 = xr_sb[:, 2 * t0:2 * t1].rearrange("p (t c) -> p t c", c=2)[:, :, 1:2]
                v03 = v0[:, :Wc].rearrange("p (t d) -> p t d", d=d)
                v13 = v1[:, :Wc].rearrange("p (t d) -> p t d", d=d)
                nc.vector.tensor_tensor(out=v03, in0=v03,
                                        in1=xr0.to_broadcast([128, ntc, d]), op=ALU.add)
                nc.vector.tensor_tensor(out=v13, in0=v13,
                                        in1=xr1.to_broadcast([128, ntc, d]), op=ALU.add)

                lr0 = scr.tile([128, wmax], f32, tag="lr0")
                lr1 = scr.tile([128, wmax], f32, tag="lr1")
                nc.vector.tensor_scalar_mul(lr0[:, :Wc], v0[:, :Wc], NEG_SLOPE)
                nc.vector.tensor_tensor(out=lr0[:, :Wc], in0=lr0[:, :Wc], in1=v0[:, :Wc], op=ALU.max)
                nc.vector.tensor_scalar_mul(lr1[:, :Wc], v1[:, :Wc], NEG_SLOPE)
                nc.vector.tensor_tensor(out=lr1[:, :Wc], in0=lr1[:, :Wc], in1=v1[:, :Wc], op=ALU.max)

                alpha = scr.tile([128, wmax], f32, tag="alpha")
                nc.vector.tensor_scalar(out=lr0[:, :Wc], in0=lr0[:, :Wc], scalar1=consts["att0"],
                                        scalar2=None, op0=ALU.mult)
                nc.vector.tensor_scalar(out=lr1[:, :Wc], in0=lr1[:, :Wc], scalar1=consts["att1"],
                                        scalar2=None, op0=ALU.mult)
                nc.vector.tensor_tensor(out=alpha[:, :Wc], in0=lr0[:, :Wc], in1=lr1[:, :Wc], op=ALU.add)
                nc.vector.tensor_tensor(out=alpha[:, :Wc], in0=alpha[:, :Wc], in1=mask_c[:, :Wc], op=ALU.add)

                nc.vector.tensor_reduce(out=namax[:, t0:t1],
                                        in_=alpha[:, :Wc].rearrange("p (t d) -> p t d", d=d),
                                        axis=AX, op=ALU.max, negate=True)
                nc.vector.tensor_scalar_min(namax[:, t0:t1], namax[:, t0:t1], 30.0)

                ex = scr.tile([128, wmax], f32, tag="ex")
                for t in range(t0, t1):
                    lo = (t - t0) * d
                    nc.scalar.activation(out=ex[:, lo:lo + d], in_=alpha[:, lo:lo + d],
                                         func=ACTF.Exp,
                                         bias=namax[:, t:t + 1], scale=1.0,
                                         accum_out=denom[:, t:t + 1])

                nc.vector.tensor_tensor(out=v0[:, :Wc], in0=ex[:, :Wc], in1=u0, op=ALU.mult)
                nc.vector.tensor_tensor(out=v1[:, :Wc], in0=ex[:, :Wc], in1=u1, op=ALU.mult)
                nc.vector.tensor_reduce(out=nume0[:, t0:t1],
                                        in_=v0[:, :Wc].rearrange("p (t d) -> p t d", d=d),
                                        axis=AX, op=ALU.add)
                nc.vector.tensor_reduce(out=nume1[:, t0:t1],
                                        in_=v1[:, :Wc].rearrange("p (t d) -> p t d", d=d),
                                        axis=AX, op=ALU.add)

            # ---- Phase F: finish ----
            outsb = pp.tile([128, 2 * NT], f32)
            dn = pp.tile([128, NT], f32)
            nc.vector.tensor_scalar_add(dn[:], denom[:], 1e-16)
            o0 = outsb[:].rearrange("p (t c) -> p t c", c=2)[:, :, 0:1].rearrange("p t one -> p (t one)")
            o1 = outsb[:].rearrange("p (t c) -> p t c", c=2)[:, :, 1:2].rearrange("p t one -> p (t one)")
            if consts["bl0"] != 0.0 or consts["bl1"] != 0.0:
                tmpb = pp.tile([128, NT], f32)
                nc.vector.tensor_scalar_mul(tmpb[:], denom[:], consts["bl0"])
                nc.vector.tensor_tensor(out=nume0[:], in0=nume0[:], in1=tmpb[:], op=ALU.add)
                nc.vector.tensor_scalar_mul(tmpb[:], denom[:], consts["bl1"])
                nc.vector.tensor_tensor(out=nume1[:], in0=nume1[:], in1=tmpb[:], op=ALU.add)
            nc.vector.reciprocal(out=dn[:], in_=dn[:])
            nc.vector.tensor_tensor(out=o0, in0=nume0[:], in1=dn[:], op=ALU.mult)
            nc.vector.tensor_tensor(out=o1, in0=nume1[:], in1=dn[:], op=ALU.mult)
            if consts["bias0"] != 0.0:
                nc.vector.tensor_scalar_add(o0, o0, consts["bias0"])
            if consts["bias1"] != 0.0:
                nc.vector.tensor_scalar_add(o1, o1, consts["bias1"])

            nc.sync.dma_start(
                out=out_d[:].rearrange("(t p) c -> p t c", p=128),
                in_=outsb[:].rearrange("p (t c) -> p t c", c=2),
            )

    nc.compile()
    return nc


def kernel(**inputs) -> np.ndarray:
    from concourse.bass_utils import run_bass_kernel_spmd

    in_maps, consts, meta, node_perm = _host_prep(**inputs)
    key = (meta["T"], meta["d_t"], meta["NI"], meta["H"], tuple(sorted(consts.items())))
    if key not in _CACHE:
        _CACHE.clear()
        _CACHE[key] = _build(meta, consts)
    nc = _CACHE[key]

    res = run_bass_kernel_spmd(nc, in_maps, list(range(N_CORES)))

    out = np.zeros((N_NODES, OUT), dtype=np.float32)
    for k in range(N_CORES):
        slab = res.results[k]["outp"]
        perm = node_perm[k]
        valid = perm < NPC
        out[k * NPC + perm[valid]] = slab[valid]
    return out


# revision 68
# speedup vs baseline: 32.9495x; 32.9495x over previous
"""GATv2 message-passing kernel for 8 Trainium2 NeuronCores.

Strategy (per core; targets sharded by node range, edge routing on-chip):
  - Host: index-only preprocessing. Targets degree-sorted into rank order;
    edges laid out as [128 target-partitions x T slots] with chunk-uniform
    slot widths. Source xl values are routed on-chip (no per-edge DMA):
      ap_gather expansion (GPSIMD) -> local_scatter into a
      (src-partition x dst-partition) bucket grid -> blocked DMA-XBAR
      transpose -> local_scatter into the target-major grid.
  - Device: xl/xr via PE matmuls on the core's rank-permuted node shard;
    bf16 xl table AllGathered; alpha/softmax/numerators on DVE+ACT with
    chunk-wide 3D-AP ops; per-target segment stats along the free axis.
"""

import numpy as np

N_NODES = 100000
N_EDGES = 6400000
D_IN = 256
OUT = 2
NEG_SLOPE = 0.2
N_CORES = 8
NPC = N_NODES // N_CORES
NT = 98
NPAD = NT * 128
NTAB = NPAD * N_CORES
VPP = NTAB // 128
CLS_W = 1920
WIN_H = 14
NTC = 7

_CACHE = {}


def _host_prep(x, edge_index, edge_attr, Wl, bl, Wr, br, We, att, bias):
    import ml_dtypes

    import hostprep as hp

    src = np.asarray(edge_index[0], dtype=np.int64)
    tgt = np.asarray(edge_index[1], dtype=np.int64)
    ea = np.asarray(edge_attr, dtype=np.float32).reshape(-1)
    x = np.asarray(x, dtype=np.float32)

    maps, meta, node_perm, rank_of = hp.prep(src, tgt, ea)

    bf16 = ml_dtypes.bfloat16
    xT = np.ascontiguousarray(x.T)  # [256, N]

    # pad slots get a poison edge_attr that drives alpha to -inf:
    # alpha_pad ~ att0*lrelu(We0*E) + att1*lrelu(We1*E); pick sign(E) so it
    # is hugely negative (pad slots then get zero softmax weight).
    att_ = np.asarray(att, np.float64)
    We_ = np.asarray(We, np.float64).reshape(-1)
    def _alpha_tail(E):
        m0, m1 = We_[0] * E, We_[1] * E
        l0 = m0 if m0 > 0 else 0.2 * m0
        l1 = m1 if m1 > 0 else 0.2 * m1
        return att_[0] * l0 + att_[1] * l1
    ea_poison = None
    for s in (1.0, -1.0):
        if _alpha_tail(s * 1e30) < -1e25:
            ea_poison = s * 1e30
            break
    assert ea_poison is not None, "degenerate att/We: keep mask path"

    W4 = np.concatenate([np.asarray(Wl, np.float32), np.asarray(Wr, np.float32)], axis=1)
    w4_sb = np.concatenate([W4[0:128, :], W4[128:256, :]], axis=1).astype(bf16)  # [128, 8]

    in_maps = []
    for k in range(N_CORES):
        # rank-permuted node columns: slab row r = node with rank r
        perm = node_perm[k]
        xTk = np.zeros((D_IN, NPAD), dtype=np.float32)
        valid = perm < NPC
        xTk[:, valid] = xT[:, k * NPC + perm[valid]]
        m = maps[k]
        ea_pp = m["ea_pp"].copy()
        ea_pp[m["mask_pp"] < 0] = ea_poison
        im = {
            "xT": xTk.astype(bf16),
            "w4": w4_sb,
            "ea_pp": ea_pp.astype(bf16),
        }
        for w in range(meta["NW"]):
            im[f"apg{w}"] = m["apg_idx"][w]
            im[f"s1i{w}"] = m["s1_idx"][w]
        for j in range(meta["n_cls"]):
            im[f"s2i{j}"] = m["s2_idx"][j]
        in_maps.append(im)

    consts = dict(
        We0=float(np.asarray(We).reshape(-1)[0]), We1=float(np.asarray(We).reshape(-1)[1]),
        att0=float(np.asarray(att)[0]), att1=float(np.asarray(att)[1]),
        K0=float(np.asarray(bl)[0] + np.asarray(br)[0]),
        K1=float(np.asarray(bl)[1] + np.asarray(br)[1]),
        bl0=float(np.asarray(bl)[0]), bl1=float(np.asarray(bl)[1]),
        bias0=float(np.asarray(bias)[0]), bias1=float(np.asarray(bias)[1]),
    )
    return in_maps, consts, meta, node_perm


def _build(meta, consts):
    from concourse import bacc, mybir
    import concourse.bass as bass
    import concourse.tile as tile

    T = meta["T"]
    d_t = meta["d_t"]
    col_off = meta["col_off"]
    NW = meta["NW"]
    NI = meta["NI"]
    H = meta["H"]
    jbase = meta["jbase"]
    SH = meta["SH"]
    n_cls = meta["n_cls"]

    f32 = mybir.dt.float32
    bf16 = mybir.dt.bfloat16
    i16 = mybir.dt.int16
    AX = mybir.AxisListType.X
    ALU = mybir.AluOpType
    ACTF = mybir.ActivationFunctionType

    NImax = max(NI)
    Hmax = max(H)

    nc = bacc.Bacc("TRN2", target_bir_lowering=False, debug=False, num_devices=N_CORES,
                   dynamic_dma_scratch_size=8192)
    xT_d = nc.dram_tensor("xT", [D_IN, NPAD], bf16, kind="ExternalInput").ap()
    w4_d = nc.dram_tensor("w4", [128, 8], bf16, kind="ExternalInput").ap()
    ea_d = nc.dram_tensor("ea_pp", [128, T], bf16, kind="ExternalInput").ap()
    apg_d = [nc.dram_tensor(f"apg{w}", [128, NI[w] // 16], i16, kind="ExternalInput").ap()
             for w in range(NW)]
    s1i_d = [nc.dram_tensor(f"s1i{w}", [128, 2 * NI[w]], i16, kind="ExternalInput").ap()
             for w in range(NW)]
    s2i_d = [nc.dram_tensor(f"s2i{j}", [128, H[j] * 128], i16, kind="ExternalInput").ap()
             for j in range(n_cls)]
    out_d = nc.dram_tensor("outp", [NPAD, OUT], f32, kind="ExternalOutput").ap()

    with tile.TileContext(nc) as tc:
        with tc.tile_pool(name="persist", bufs=1) as pp, \
             tc.tile_pool(name="stream", bufs=2) as sp, \
             tc.tile_pool(name="route", bufs=1) as rp, \
             tc.tile_pool(name="idx", bufs=3) as rq, \
             tc.tile_pool(name="scratch", bufs=3) as scr, \
             tc.tile_pool(name="psum", bufs=4, space="PSUM") as psp, \
             tc.tile_pool(name="dram", bufs=1, space="DRAM") as dp:

            w4_sb = pp.tile([128, 8], bf16)
            nc.sync.dma_start(out=w4_sb[:], in_=w4_d[:])

            xl_slab_sb = pp.tile([128, 2 * NT], bf16)
            xr_sb = pp.tile([128, 2 * NT], f32)

            # ---- Phase A: xl/xr for this core's node shard (rank order) ----
            NBC = 16  # node blocks per x-stream chunk
            for cb in range(0, NT, NBC):
                ce = min(cb + NBC, NT)
                xa = sp.tile([128, 128 * NBC], bf16, tag="xa")
                xb = sp.tile([128, 128 * NBC], bf16, tag="xb")
                nc.scalar.dma_start(out=xa[:, :128 * (ce - cb)],
                                     in_=xT_d[0:128, 128 * cb:128 * ce])
                nc.sync.dma_start(out=xb[:, :128 * (ce - cb)],
                                  in_=xT_d[128:256, 128 * cb:128 * ce])
                ps = psp.tile([128, 4 * NBC], f32)
                for nb in range(cb, ce):
                    o = 128 * (nb - cb)
                    po = 4 * (nb - cb)
                    nc.tensor.matmul(out=ps[:, po:po + 4], lhsT=xa[:, o:o + 128],
                                     rhs=w4_sb[:, 0:4], start=True, stop=False)
                    nc.tensor.matmul(out=ps[:, po:po + 4], lhsT=xb[:, o:o + 128],
                                     rhs=w4_sb[:, 4:8], start=False, stop=True)
                # batched PSUM evictions: [128, nbc, 2] strided views
                nbc = ce - cb
                ps3 = ps[:, :4 * nbc].rearrange("p (t c) -> p t c", c=2)
                nc.vector.tensor_copy(
                    out=xl_slab_sb[:, 2 * cb:2 * ce].rearrange("p (t c) -> p t c", c=2),
                    in_=ps3[:, 0::2, :])
                nc.vector.tensor_copy(
                    out=xr_sb[:, 2 * cb:2 * ce].rearrange("p (t c) -> p t c", c=2),
                    in_=ps3[:, 1::2, :])

            xl_slab_d = dp.tile([NPAD, 2], bf16)
            nc.sync.dma_start(
                out=xl_slab_d[:].rearrange("(t p) c -> p t c", p=128),
                in_=xl_slab_sb[:].rearrange("p (t c) -> p t c", c=2),
            )

            xl_full = dp.tile([NTAB, 2], bf16)
            nc.gpsimd.collective_compute(
                "AllGather",
                mybir.AluOpType.bypass,
                replica_groups=[list(range(N_CORES))],
                ins=[xl_slab_d.opt()],
                outs=[xl_full.opt()],
            )
            tok = pp.tile([128, 2], bf16)
            nc.sync.dma_start(out=tok[:], in_=xl_full[NTAB - 128:NTAB, :])
            tc.strict_bb_all_engine_barrier()

            # xl table: partition q holds nodes [784q, 784(q+1)) interleaved ch
            tab_sb = pp.tile([128, 2 * VPP], bf16)
            nc.sync.dma_start(
                out=tab_sb[:].rearrange("q (v c) -> q v c", c=2),
                in_=xl_full[:].rearrange("(q v) c -> q v c", q=128),
            )

            # ---- Phase B/D: expansion + scatter#1 into bucket grid ----
            bucket = pp.tile([128, SH * 128], bf16)

            def emit_window(w):
                apg_sb = rq.tile([128, NImax // 16], i16, tag="apg")
                s1i_sb = rq.tile([128, 2 * NImax], i16, tag="s1i")
                nc.sync.dma_start(out=apg_sb[:, :NI[w] // 16], in_=apg_d[w][:])
                nc.sync.dma_start(out=s1i_sb[:, :2 * NI[w]], in_=s1i_d[w][:])
                exp_sb = rp.tile([128, 2 * NImax], bf16, tag="exp")
                nc.gpsimd.ap_gather(
                    out_ap=exp_sb[:, :2 * NI[w]].rearrange("p (i c) -> p i c", c=2),
                    in_ap=tab_sb[:].rearrange("q (v c) -> q v c", c=2),
                    idxs_ap=apg_sb[:, :NI[w] // 16],
                    channels=128,
                    num_elems=VPP,
                    d=2,
                    num_idxs=NI[w],
                )
                lo = WIN_H * w * 128
                ne = min(WIN_H * 128, SH * 128 - lo)
                nc.gpsimd.local_scatter(
                    out_ap=bucket[:, lo:lo + ne],
                    data_ap=exp_sb[:, :2 * NI[w]],
                    idxs_ap=s1i_sb[:, :2 * NI[w]],
                    channels=128,
                    num_elems=ne,
                    num_idxs=2 * NI[w],
                )

            # ---- Phase D'/E: transpose + scatter#2 per class, interleaved
            #      with alpha chunks over the completed uF prefix ----
            uF = pp.tile([128, 2 * T], bf16)
            namax = pp.tile([128, NT], f32)
            denom = pp.tile([128, NT], f32)
            nume0 = pp.tile([128, NT], f32)
            nume1 = pp.tile([128, NT], f32)

            SINGLES = 7
            chunks = [(t, t + 1, col_off[t], col_off[t] + d_t[t]) for t in range(SINGLES)]
            for t0 in range(SINGLES, NT, NTC):
                t1 = min(t0 + NTC, NT)
                chunks.append((t0, t1, col_off[t0], col_off[t0] + (t1 - t0) * d_t[t0]))
            wmax = max(c[3] - c[2] for c in chunks)
            next_chunk = [0]

            def emit_transpose(j):
                tj = rq.tile([128, Hmax * 128], bf16, tag="tj")
                nc.sync.dma_start(
                    out=tj[:, :H[j] * 128].rearrange("p (b q) -> p b q", q=128),
                    in_=bucket[:, jbase[j] * 128:(jbase[j] + H[j]) * 128],
                    transpose=True,
                )
                s2i_sb = rp.tile([128, Hmax * 128], i16, tag="s2i")
                nc.sync.dma_start(out=s2i_sb[:, :H[j] * 128], in_=s2i_d[j][:])
                return tj, s2i_sb

            def emit_s2(j, tj, s2i_sb):
                lo = CLS_W * j
                ne = min(CLS_W, 2 * T - lo)
                nc.gpsimd.local_scatter(
                    out_ap=uF[:, lo:lo + ne],
                    data_ap=tj[:, :H[j] * 128],
                    idxs_ap=s2i_sb[:, :H[j] * 128],
                    channels=128,
                    num_elems=ne,
                    num_idxs=H[j] * 128,
                )

            def emit_alpha(t0, t1, c0, c1):
                Wc = c1 - c0
                ntc = t1 - t0
                d = d_t[t0]
                ea_c = sp.tile([128, wmax], bf16, tag="ea")
                nc.sync.dma_start(out=ea_c[:, :Wc], in_=ea_d[:, c0:c1])

                uch = uF[:, 2 * c0:2 * c1].rearrange("p (w c) -> p w c", c=2)
                u0 = uch[:, :, 0:1].rearrange("p w one -> p (w one)")
                u1 = uch[:, :, 1:2].rearrange("p w one -> p (w one)")

                v0 = scr.tile([128, wmax], f32, tag="v0")
                v1 = scr.tile([128, wmax], f32, tag="v1")
                nc.scalar.activation(out=v0[:, :Wc], in_=ea_c[:, :Wc], func=ACTF.Copy,
                                     bias=consts["K0"], scale=consts["We0"])
                nc.scalar.activation(out=v1[:, :Wc], in_=ea_c[:, :Wc], func=ACTF.Copy,
                                     bias=consts["K1"], scale=consts["We1"])
                nc.vector.tensor_tensor(out=v0[:, :Wc], in0=v0[:, :Wc], in1=u0, op=ALU.add)
                nc.vector.tensor_tensor(out=v1[:, :Wc], in0=v1[:, :Wc], in1=u1, op=ALU.add)

                # += xr (per-partition per-tile broadcast), chunk-wide 3D AP
                xr0 = xr_sb[:, 2 * t0:2 * t1].rearrange("p (t c) -> p t c", c=2)[:, :, 0:1]
                xr1 = xr_sb[:, 2 * t0:2 * t1].rearrange("p (t c) -> p t c", c=2)[:, :, 1:2]
                v03 = v0[:, :Wc].rearrange("p (t d) -> p t d", d=d)
                v13 = v1[:, :Wc].rearrange("p (t d) -> p t d", d=d)
                nc.vector.tensor_tensor(out=v03, in0=v03,
                                        in1=xr0.to_broadcast([128, ntc, d]), op=ALU.add)
                nc.vector.tensor_tensor(out=v13, in0=v13,
                                        in1=xr1.to_broadcast([128, ntc, d]), op=ALU.add)

                # a_ch = att_ch * lrelu(v_ch) = (max if att>=0 else min)(0.2*att*v, att*v)
                lr0 = scr.tile([128, wmax], f32, tag="lr0")
                lr1 = scr.tile([128, wmax], f32, tag="lr1")
                op0 = ALU.max if consts["att0"] >= 0 else ALU.min
                op1 = ALU.max if consts["att1"] >= 0 else ALU.min
                nc.scalar.activation(out=lr0[:, :Wc], in_=v0[:, :Wc], func=ACTF.Copy,
                                     bias=0.0, scale=NEG_SLOPE * consts["att0"])
                nc.scalar.activation(out=v0[:, :Wc], in_=v0[:, :Wc], func=ACTF.Copy,
                                     bias=0.0, scale=consts["att0"])
                nc.vector.tensor_tensor(out=lr0[:, :Wc], in0=lr0[:, :Wc], in1=v0[:, :Wc], op=op0)
                nc.scalar.activation(out=lr1[:, :Wc], in_=v1[:, :Wc], func=ACTF.Copy,
                                     bias=0.0, scale=NEG_SLOPE * consts["att1"])
                nc.scalar.activation(out=v1[:, :Wc], in_=v1[:, :Wc], func=ACTF.Copy,
                                     bias=0.0, scale=consts["att1"])
                nc.vector.tensor_tensor(out=lr1[:, :Wc], in0=lr1[:, :Wc], in1=v1[:, :Wc], op=op1)

                alpha = lr0  # reuse: lr0 dead after this add
                nc.vector.tensor_tensor(out=alpha[:, :Wc], in0=lr0[:, :Wc], in1=lr1[:, :Wc], op=ALU.add)

                nc.vector.tensor_reduce(out=namax[:, t0:t1],
                                        in_=alpha[:, :Wc].rearrange("p (t d) -> p t d", d=d),
                                        axis=AX, op=ALU.max, negate=True)
                nc.vector.tensor_scalar_min(namax[:, t0:t1], namax[:, t0:t1], 30.0)

                ex = lr1  # reuse: lr1 dead after the alpha add
                for t in range(t0, t1):
                    lo = (t - t0) * d
                    nc.scalar.activation(out=ex[:, lo:lo + d], in_=alpha[:, lo:lo + d],
                                         func=ACTF.Exp,
                                         bias=namax[:, t:t + 1], scale=1.0,
                                         accum_out=denom[:, t:t + 1])

                nc.vector.tensor_tensor(out=v0[:, :Wc], in0=ex[:, :Wc], in1=u0, op=ALU.mult)
                nc.vector.tensor_tensor(out=v1[:, :Wc], in0=ex[:, :Wc], in1=u1, op=ALU.mult)
                nc.vector.tensor_reduce(out=nume0[:, t0:t1],
                                        in_=v0[:, :Wc].rearrange("p (t d) -> p t d", d=d),
                                        axis=AX, op=ALU.add)
                nc.vector.tensor_reduce(out=nume1[:, t0:t1],
                                        in_=v1[:, :Wc].rearrange("p (t d) -> p t d", d=d),
                                        axis=AX, op=ALU.add)

            next_cls = 0
            pending = []

            def flush_pending():
                while pending:
                    j, tj, s2i_sb = pending.pop(0)
                    emit_s2(j, tj, s2i_sb)
                    covered = min(CLS_W * (j + 1), 2 * T)
                    while next_chunk[0] < len(chunks):
                        (t0, t1, c0, c1) = chunks[next_chunk[0]]
                        if 2 * c1 > covered:
                            break
                        emit_alpha(t0, t1, c0, c1)
                        next_chunk[0] += 1

            for w in range(NW):
                flush_pending()   # scatter#2 deferred one window past its transpose
                emit_window(w)
                blocks_done = min(WIN_H * (w + 1), SH)
                while next_cls < n_cls and jbase[next_cls] + H[next_cls] <= blocks_done:
                    pending.append((next_cls, *emit_transpose(next_cls)))
                    next_cls += 1
            while next_cls < n_cls:
                pending.append((next_cls, *emit_transpose(next_cls)))
                next_cls += 1
            flush_pending()
            while next_chunk[0] < len(chunks):
                emit_alpha(*chunks[next_chunk[0]])
                next_chunk[0] += 1

            # ---- Phase F: finish ----
            outsb = pp.tile([128, 2 * NT], f32)
            dn = pp.tile([128, NT], f32)
            nc.vector.tensor_scalar_add(dn[:], denom[:], 1e-16)
            o0 = outsb[:].rearrange("p (t c) -> p t c", c=2)[:, :, 0:1].rearrange("p t one -> p (t one)")
            o1 = outsb[:].rearrange("p (t c) -> p t c", c=2)[:, :, 1:2].rearrange("p t one -> p (t one)")
            if consts["bl0"] != 0.0 or consts["bl1"] != 0.0:
                tmpb = pp.tile([128, NT], f32)
                nc.vector.tensor_scalar_mul(tmpb[:], denom[:], consts["bl0"])
                nc.vector.tensor_tensor(out=nume0[:], in0=nume0[:], in1=tmpb[:], op=ALU.add)
                nc.vector.tensor_scalar_mul(tmpb[:], denom[:], consts["bl1"])
                nc.vector.tensor_tensor(out=nume1[:], in0=nume1[:], in1=tmpb[:], op=ALU.add)
            nc.vector.reciprocal(out=dn[:], in_=dn[:])
            nc.vector.tensor_tensor(out=o0, in0=nume0[:], in1=dn[:], op=ALU.mult)
            nc.vector.tensor_tensor(out=o1, in0=nume1[:], in1=dn[:], op=ALU.mult)
            if consts["bias0"] != 0.0:
                nc.vector.tensor_scalar_add(o0, o0, consts["bias0"])
            if consts["bias1"] != 0.0:
                nc.vector.tensor_scalar_add(o1, o1, consts["bias1"])

            nc.sync.dma_start(
                out=out_d[:].rearrange("(t p) c -> p t c", p=128),
                in_=outsb[:].rearrange("p (t c) -> p t c", c=2),
            )

    nc.compile()
    return nc


def kernel(**inputs) -> np.ndarray:
    from concourse.bass_utils import run_bass_kernel_spmd

    in_maps, consts, meta, node_perm = _host_prep(**inputs)
    key = (meta["T"], meta["d_t"], meta["NI"], meta["H"], tuple(sorted(consts.items())))
    if key not in _CACHE:
        _CACHE.clear()
        _CACHE[key] = _build(meta, consts)
    nc = _CACHE[key]

    res = run_bass_kernel_spmd(nc, in_maps, list(range(N_CORES)))

    out = np.zeros((N_NODES, OUT), dtype=np.float32)
    for k in range(N_CORES):
        slab = res.results[k]["outp"]
        perm = node_perm[k]
        valid = perm < NPC
        out[k * NPC + perm[valid]] = slab[valid]
    return out
